# revision 20
# baseline (speedup 1.0000x reference)
"""Trainium2 Bass kernel for nn_MultiHeadFast (multi-head attention with
softmax over the QUERY axis).

Math (faithful to the reference):
  qkv = x @ Ws;  per (b,h):  S[q,k] = Q.K^T,  causal mask k<=q,
  P = softmax_over_q(S * T^-0.5),  out = P @ V.

v2 design (PE-minimal):
  - Sharding: 8 cores = 2 batches x 4 head-groups.  Core c owns batch c//4
    and 4 heads, processed as 2 passes of 2 heads (PSUM limit).
  - Host passes x^T (bf16) and the per-core Ws column slice (bf16), so the
    device does ZERO input transposes.  V is produced token-major via an
    XBAR DMA transpose (out[p,i,d] = in[d, i*128+p]), not the PE.
  - S^T is computed only on the live causal region (q >= 128*ktile) in
    <=512-col matmuls; exp (query-axis softmax numerator) on ScalarE;
    per-key normalizers via VectorE free-axis reduce of the bf16 strip.
  - out^T[d,q] = sum_k V_norm^T P^T accumulates in PSUM and is DMA'd out
    transposed; the host does the final cheap (128,2048)->(2048,128)
    transpose.  Pipeline: [QKV pass0] [S/exp pass0 + QKV pass1 on PE]
    [S/exp pass1 + PV pass0 + PV pass1 slab-major] [PV1 tail].
"""

import numpy as np
import ml_dtypes
from contextlib import ExitStack

import concourse.bass as bass
import concourse.mybir as mybir
import concourse.tile as tile
from concourse import bacc
from concourse.bass_utils import run_bass_kernel_spmd

B, T, E = 2, 2048, 1024
H, D = 16, 64
NCORES = 8
P = 128
EK = E // P           # 8 contraction blocks
KT = T // P           # 16 key tiles per core-batch
NS = T // 512         # 4 query slabs
DT = mybir.dt.bfloat16
F32 = mybir.dt.float32
SCALE = float(T) ** -0.5
NEG = -1e30


def live(k):
    return T - P * k


def build_kernel():
    nc = bacc.Bacc("TRN2", target_bir_lowering=False, debug=False)
    x_dram = nc.dram_tensor("x", (E, T), DT, kind="ExternalInput")       # x^T
    w_dram = nc.dram_tensor("wsl", (E, 768), DT, kind="ExternalInput")
    out_dram = nc.dram_tensor("out", (2, P, T), F32, kind="ExternalOutput")

    with tile.TileContext(nc) as tc, ExitStack() as ctx:
        const = ctx.enter_context(tc.tile_pool(name="const", bufs=1))
        xp = ctx.enter_context(tc.tile_pool(name="xp", bufs=1))
        qkvp = ctx.enter_context(tc.tile_pool(name="qkvp", bufs=1))
        strips = ctx.enter_context(tc.tile_pool(name="strips", bufs=1))
        small = ctx.enter_context(tc.tile_pool(name="small", bufs=1))
        ps = ctx.enter_context(tc.tile_pool(name="ps", bufs=1, space="PSUM"))

        # ---- constants ----
        zeros_bf = const.tile([P, P], DT, name="zeros_bf")
        nc.gpsimd.memset(zeros_bf[:], 0.0)
        # diagmask[p, f] = 0 if f >= p else NEG (keys on partitions, q free)
        diagmask = const.tile([P, P], F32, name="diagmask")
        nc.gpsimd.memset(diagmask[:], 0.0)
        nc.gpsimd.affine_select(
            out=diagmask[:],
            in_=diagmask[:],
            compare_op=mybir.AluOpType.is_ge,
            fill=NEG,
            base=0,
            pattern=[[1, P]],
            channel_multiplier=-1,
        )

        # ---- input DMAs ----
        wsl = qkvp.tile([P, EK, 768], DT, name="wsl")
        nc.sync.dma_start(wsl[:], w_dram.rearrange("(eo ei) f -> ei eo f", ei=P))
        xT = xp.tile([P, EK, T], DT, name="xT")
        for s in range(NS):
            nc.sync.dma_start(
                xT[:, :, 512 * s : 512 * (s + 1)],
                x_dram[:, 512 * s : 512 * (s + 1)].rearrange(
                    "(eo ei) t -> ei eo t", ei=P
                ),
            )

        # ---- per-pass tensors ----
        qt = [qkvp.tile([P, T], DT, name=f"qt{p}") for p in range(2)]
        kt = [qkvp.tile([P, T], DT, name=f"kt{p}") for p in range(2)]
        vt = [qkvp.tile([P, T], DT, name=f"vt{p}") for p in range(2)]
        vnat = [qkvp.tile([P, KT, P], DT, name=f"vn{p}") for p in range(2)]
        vp_all = [qkvp.tile([P, KT, 2, D], DT, name=f"vp{p}") for p in range(2)]
        rsum_all = [qkvp.tile([P, 2 * KT], F32, name=f"rs{p}") for p in range(2)]

        def qkv_unit(p, m, s):
            """One 512-token slab of Q^T/K^T/V^T (m=0/1/2) for pass p.
            Q and K run as fp8 DoubleRow (contraction 256 per matmul);
            V stays bf16 for accuracy (its error passes straight through)."""
            dst = (qt, kt, vt)[m][p]
            mm = ps.tile([P, 512], F32, tag="b512", bufs=4, name="qkv_ps")
            for e in range(EK):
                nc.tensor.matmul(
                    mm[:],
                    lhsT=wsl[:, e, 256 * m + P * p : 256 * m + P * (p + 1)],
                    rhs=xT[:, e, 512 * s : 512 * (s + 1)],
                    start=(e == 0),
                    stop=(e == EK - 1),
                )
            nc.vector.tensor_copy(dst[:, 512 * s : 512 * (s + 1)], mm[:])
            if m == 2:
                # V natural layout for this slab: vnat[p_, i, d] = vt[d, i*128+p_]
                nc.sync.dma_start_transpose(
                    vnat[p][:, 4 * s : 4 * (s + 1), :],
                    vt[p][:, 512 * s : 512 * (s + 1)],
                )

        def s_exp_pair(p, k, strips_kh):
            """S^T matmuls + mask + exp + normalizer for both heads of a
            (pass, ktile).  Head h uses PE row-groups h*64..h*64+63, so
            interleaving heads lets LDWEIGHTS overlap in-flight matmuls."""
            L = live(k)
            q0 = P * k
            parts = {0: [], 1: []}
            for c in range(0, L, 1024):
                cw = min(1024, L - c)
                sps = {}
                for h in range(2):
                    sps[h] = ps.tile([P, 1024], F32, tag="sps", bufs=2, name="sps")
                for so in range(0, cw, 512):
                    w = min(512, cw - so)
                    for h in range(2):
                        nc.tensor.matmul(
                            sps[h][:, so : so + w],
                            lhsT=kt[p][h * D : (h + 1) * D, q0 : q0 + P],
                            rhs=qt[p][h * D : (h + 1) * D,
                                      q0 + c + so : q0 + c + so + w],
                            start=True,
                            stop=True,
                        )
                for h in range(2):
                    if c == 0:
                        nc.vector.tensor_add(sps[h][:, 0:P], sps[h][:, 0:P], diagmask[:])
                    acc = small.tile([P, 1], F32, tag="acc", bufs=8, name="acc")
                    nc.scalar.activation(
                        strips_kh[h][:, c : c + cw],
                        sps[h][:, :cw],
                        mybir.ActivationFunctionType.Exp,
                        scale=SCALE,
                        accum_out=acc[:],
                    )
                    parts[h].append(acc)
            for h in range(2):
                if len(parts[h]) == 1:
                    ssum = parts[h][0]
                else:
                    ssum = small.tile([P, 1], F32, tag="acc", bufs=8, name="ssum")
                    nc.vector.tensor_add(ssum[:], parts[h][0][:], parts[h][1][:])
                nc.vector.reciprocal(
                    rsum_all[p][:, 2 * k + h : 2 * k + h + 1], ssum[:]
                )

        def pv_mms(p, k, h, strip, pv, j, last):
            """PV contribution of (pass, ktile, head) to out^T slab j."""
            j0 = k // 4
            if j == j0:
                coff = P * (k % 4)
                nc.tensor.matmul(
                    pv[h * D : (h + 1) * D, coff:512],
                    lhsT=vp_all[p][:, k, h, :],
                    rhs=strip[:, 0 : 512 - coff],
                    start=False,
                    stop=last,
                    skip_group_check=True,
                )
            else:
                c = 512 * j - P * k
                nc.tensor.matmul(
                    pv[h * D : (h + 1) * D, :],
                    lhsT=vp_all[p][:, k, h, :],
                    rhs=strip[:, c : c + 512],
                    start=False,
                    stop=last,
                    skip_group_check=True,
                )

        def dma_out_slab(p, j, pv):
            ob = strips.tile([P, 512], F32, tag="outb", bufs=2, name="outb")
            nc.vector.tensor_copy(ob[:], pv[:])
            nc.sync.dma_start(out_dram[p, :, 512 * j : 512 * (j + 1)], ob[:])

        def new_pv_bank():
            pv = ps.tile([P, 512], F32, tag="b512", bufs=4, name="pv")
            nc.tensor.matmul(
                pv[:],
                lhsT=zeros_bf[:],
                rhs=xT[:, 0, 0:512],
                start=True,
                stop=False,
                skip_group_check=True,
            )
            return pv

        stripd = [{}, {}]
        pvd = {}

        def vp_scale(p, k0, k1):
            for kk in range(k0, k1):
                for h in range(2):
                    nc.vector.tensor_scalar_mul(
                        vp_all[p][:, kk, h, :],
                        vnat[p][:, kk, :][:, h * D : (h + 1) * D],
                        rsum_all[p][:, 2 * kk + h : 2 * kk + h + 1],
                    )

        def pv_start(p, j):
            """Open out^T slab j: zero the bank + contributions from k < 4j
            (their exps are already done, so this is pure PE filler)."""
            pvd[(p, j)] = new_pv_bank()
            for kk in range(4 * j):
                for h in range(2):
                    pv_mms(p, kk, h, stripd[p][(kk, h)], pvd[(p, j)], j, last=False)

        def pv_fin(p, j):
            """Close slab j: contributions from its own k-quad + DMA out."""
            for kk in range(4 * j, 4 * j + 4):
                for h in range(2):
                    pv_mms(p, kk, h, stripd[p][(kk, h)], pvd[(p, j)], j,
                           last=(kk == 4 * j + 3 and h == 1))
            dma_out_slab(p, j, pvd[(p, j)])

        # PE filler units (QKV slabs) spread through the exp stream so the
        # tensor engine never idles long enough for the HAM clock-gate to
        # re-throttle.  Each entry is (pass, m, slab), emitted after the
        # s_exp_pair of the listed iteration.
        FILL = {
            0: {0: (0, 2, 0), 1: (0, 1, 1), 2: (0, 2, 1), 3: (0, 1, 2),
                4: (1, 0, 0), 5: (0, 1, 3), 6: (0, 2, 2), 7: (1, 0, 1),
                8: (0, 2, 3), 9: (1, 0, 2), 10: (1, 0, 3), 11: (1, 1, 0)},
            1: {0: (1, 2, 0), 1: (1, 1, 1), 2: (1, 2, 1), 3: (1, 1, 2),
                4: (1, 2, 2), 5: (1, 1, 3), 6: (1, 2, 3)},
        }

        # prelude: everything S(0, k=0..3) needs
        for s in range(NS):
            qkv_unit(0, 0, s)
        qkv_unit(0, 1, 0)

        for p in range(2):
            for k in range(KT):
                nbuf = 4 if k < 2 else 2
                sts = [
                    strips.tile([P, live(k)], DT, tag=f"s{k}", bufs=nbuf,
                                name=f"s{p}_{k}")
                    for _ in range(2)
                ]
                s_exp_pair(p, k, sts)
                for h in range(2):
                    stripd[p][(k, h)] = sts[h]
                if k in FILL[p]:
                    qkv_unit(*FILL[p][k])
                if k % 4 == 0:
                    j = k // 4
                    if j >= 1:
                        vp_scale(p, 4 * (j - 1), 4 * j)
                        pv_fin(p, j - 1)
                    pv_start(p, j)
            vp_scale(p, 12, 16)
            pv_fin(p, 3)

    nc.compile()
    return nc


def prep_in_maps(x, Ws):
    x = np.asarray(x, np.float32)
    Ws = np.asarray(Ws, np.float32)
    in_maps = []
    for c in range(NCORES):
        b = c // 4
        xT = np.ascontiguousarray(x[b].T).astype(ml_dtypes.bfloat16)
        blocks = []
        for m in range(3):
            for p in range(2):
                g = (c % 4) * 4 + 2 * p
                blocks.append(Ws[:, m * E + D * g : m * E + D * g + 2 * D])
        wsl = np.concatenate(blocks, axis=1).astype(ml_dtypes.bfloat16)
        in_maps.append({"x": xT, "wsl": np.ascontiguousarray(wsl)})
    return in_maps


def assemble(results):
    out = np.empty((B, T, H * D), np.float32)
    for c in range(NCORES):
        r = np.asarray(results[c]["out"], np.float32)
        b = c // 4
        for p in range(2):
            for h in range(2):
                g = (c % 4) * 4 + 2 * p + h
                out[b, :, D * g : D * (g + 1)] = r[p, D * h : D * (h + 1), :].T
    return out


_NC_CACHE = None


def kernel(x: np.ndarray, Ws: np.ndarray) -> np.ndarray:
    global _NC_CACHE
    if _NC_CACHE is None:
        _NC_CACHE = build_kernel()
    nc = _NC_CACHE
    res = run_bass_kernel_spmd(nc, prep_in_maps(x, Ws), core_ids=list(range(NCORES)))
    return assemble(res.results)
